# revision 4
# baseline (speedup 1.0000x reference)
"""MoE feed-forward v3: expert-parallel masked-dense in bf16.

Each core c computes expert c's FFN over ALL 8192 tokens in transposed
(channels-on-partitions) layout, streaming token chunks of 256:
  - router (fp32 matmuls, exact top-2 softmax weight for expert c; zero
    for unrouted tokens) fused into the same chunk loop,
  - mm1 (bf16, W1 resident in SBUF) + gelu -> h.T chunk,
  - mm2 (bf16, W2 resident in SBUF), scale columns by the combine weight
    (broadcast via a DRAM round-trip), store y.T chunk bf16.
A bf16 ReduceScatter over the [1024, 8192] y.T buffers gives each core a
[128, 8192] channel shard; the host concatenates, transposes, casts.

No gathers/scatters/compaction: 4x the matmul FLOPs of a compacted
design, but every DMA is large+regular, the PE runs uninterrupted bf16
matmuls (1 cyc/row vs fp32's 4), and there is no SWDGE indirect traffic.
"""

import numpy as np

B, T, DIM, FF, E = 4, 2048, 1024, 4096, 8
N = B * T                # 8192
P = 128
KC = DIM // P            # 8
FFC = FF // P            # 32
DC = DIM // P            # 8
TW = 256                 # token chunk width
NCH = N // TW            # 32
G = TW // P              # 2 groups per chunk

_cache = {}


def _legalize_waits(nc):
    """Move Tile-attached semaphore waits onto standalone EventSemaphore
    instructions — this walrus build rejects instructions carrying attached
    sync waits (LDWEIGHTS/Drain with >=2 fail to encode)."""
    import concourse.mybir as mybir

    moved = 0
    for bb in nc.main_func.blocks:
        insts = bb.instructions
        out = []
        for ins in insts:
            si = ins.sync_info
            waits = list(si.on_wait) if si is not None else []
            if waits:
                for k, w in enumerate(waits):
                    car = mybir.InstEventSemaphore(
                        name=f"{ins.name}_wt{k}", ins=[], outs=[]
                    )
                    car.engine = ins.engine
                    csi = car.sync_info
                    if csi is None:
                        csi = mybir.SyncInfo(on_wait=[], on_update=[])
                    csi.on_wait = [w]
                    car.sync_info = csi
                    out.append(car)
                    moved += 1
                si.on_wait = []
                ins.sync_info = si
            out.append(ins)
        while len(insts):
            insts.pop()
        for x in out:
            insts.append(x)
    return moved


def _build(collective=True):
    import concourse.bass as bass
    import concourse.mybir as mybir
    import concourse.tile as tile

    fp32 = mybir.dt.float32
    bf16 = mybir.dt.bfloat16
    AX = mybir.AxisListType
    ALU = mybir.AluOpType
    ACT = mybir.ActivationFunctionType

    nc = bass.Bass()
    xTf = nc.declare_dram_parameter("xTf", [DIM, N], fp32, isOutput=False)
    xTb = nc.declare_dram_parameter("xTb", [DIM, N], bf16, isOutput=False)
    wrt = nc.declare_dram_parameter("wrt", [DIM, E], fp32, isOutput=False)
    w1 = nc.declare_dram_parameter("w1", [DIM, FF], bf16, isOutput=False)
    w2 = nc.declare_dram_parameter("w2", [FF, DIM], bf16, isOutput=False)
    esel = nc.declare_dram_parameter("esel", [P, E], fp32, isOutput=False)
    eye = nc.declare_dram_parameter("eye", [P, P], fp32, isOutput=False)
    out_ext = nc.declare_dram_parameter("out", [P, N], bf16, isOutput=True)

    with tile.TileContext(nc) as tc:
        with (
            tc.tile_pool(name="const", bufs=1) as constp,
            tc.tile_pool(name="wres", bufs=1) as wresp,
            tc.tile_pool(name="xf", bufs=2) as xfp,
            tc.tile_pool(name="xb", bufs=2) as xbp,
            tc.tile_pool(name="ht", bufs=1) as htp,
            tc.tile_pool(name="yt", bufs=2) as ytp,
            tc.tile_pool(name="rt", bufs=4) as rtp,
            tc.tile_pool(name="wbp", bufs=2) as wbp,
            tc.tile_pool(name="ps_l", bufs=2, space="PSUM") as ps_l,
            tc.tile_pool(name="ps_t", bufs=2, space="PSUM") as ps_t,
            tc.tile_pool(name="ps_h", bufs=2, space="PSUM") as ps_h,
            tc.tile_pool(name="ps_y", bufs=2, space="PSUM") as ps_y,
            tc.tile_pool(name="dram", bufs=1, space="DRAM") as dram,
            tc.tile_pool(name="dramw", bufs=2, space="DRAM") as dramw,
        ):
            # constants + resident weights
            wrt_sb = constp.tile([P, KC, E], fp32)
            nc.sync.dma_start(wrt_sb[:], wrt.rearrange("(kc p) e -> p kc e", p=P))
            esel_sb = constp.tile([P, E], fp32)
            nc.sync.dma_start(esel_sb[:], esel[:, :])
            eye_sb = constp.tile([P, P], fp32)
            nc.sync.dma_start(eye_sb[:], eye[:, :])
            w1_sb = wresp.tile([P, KC, FF], bf16)
            nc.sync.dma_start(w1_sb[:], w1.rearrange("(kc p) f -> p kc f", p=P))
            w2_sb = wresp.tile([P, FFC, DIM], bf16)
            nc.sync.dma_start(w2_sb[:], w2.rearrange("(fc p) d -> p fc d", p=P))

            yt2 = dram.tile([DIM, N], bf16)

            for c in range(NCH):
                ts = c * TW
                xf_sb = xfp.tile([P, KC, TW], fp32)
                nc.sync.dma_start(
                    xf_sb[:], xTf[:, ts:ts + TW].rearrange("(kc p) n -> p kc n", p=P)
                )
                xb_sb = xbp.tile([P, KC, TW], bf16)
                nc.sync.dma_start(
                    xb_sb[:], xTb[:, ts:ts + TW].rearrange("(kc p) n -> p kc n", p=P)
                )

                # ---- router: top-2 softmax weight for this core's expert ----
                l3 = rtp.tile([P, G, E], fp32)
                for g in range(G):
                    psl = ps_l.tile([P, E], fp32)
                    for kc in range(KC):
                        nc.tensor.matmul(
                            psl[:],
                            xf_sb[:, kc, g * P:(g + 1) * P],
                            wrt_sb[:, kc, :],
                            start=(kc == 0),
                            stop=(kc == KC - 1),
                        )
                    nc.scalar.copy(l3[:, g, :], psl[:])
                m1 = rtp.tile([P, G], fp32)
                nc.vector.reduce_max(m1[:], l3[:], axis=AX.X)
                nm1 = rtp.tile([P, G], fp32)
                nc.scalar.mul(nm1[:], m1[:], -1.0)
                lg = rtp.tile([P, G, E], fp32)
                nc.vector.tensor_tensor(
                    lg[:], l3[:], nm1[:, :].unsqueeze(2).broadcast_to((P, G, E)),
                    ALU.add,
                )
                msk = rtp.tile([P, G, E], fp32)
                nc.vector.tensor_scalar(msk[:], lg[:], 0.0, None, ALU.is_ge)
                lmk = rtp.tile([P, G, E], fp32)
                nc.vector.tensor_scalar(lmk[:], msk[:], -1e30, None, ALU.mult)
                nc.vector.tensor_tensor(lmk[:], lmk[:], lg[:], ALU.add)
                m2 = rtp.tile([P, G], fp32)
                nc.vector.reduce_max(m2[:], lmk[:], axis=AX.X)
                el = rtp.tile([P, G, E], fp32)
                nc.scalar.activation(el[:], lg[:], ACT.Exp)
                em2 = rtp.tile([P, G], fp32)
                nc.scalar.activation(em2[:], m2[:], ACT.Exp)
                den = rtp.tile([P, G], fp32)
                nc.scalar.add(den[:], em2[:], 1.0)
                rden = rtp.tile([P, G], fp32)
                nc.vector.reciprocal(rden[:], den[:])
                sel = rtp.tile([P, G, E], fp32)
                nc.vector.tensor_tensor(
                    sel[:], lg[:], m2[:, :].unsqueeze(2).broadcast_to((P, G, E)),
                    ALU.is_ge,
                )
                w8 = rtp.tile([P, G, E], fp32)
                nc.vector.tensor_tensor(w8[:], el[:], sel[:], ALU.mult)
                nc.vector.tensor_tensor(
                    w8[:], w8[:], rden[:, :].unsqueeze(2).broadcast_to((P, G, E)),
                    ALU.mult,
                )
                nc.vector.tensor_tensor(
                    w8[:], w8[:], esel_sb[:, :].unsqueeze(1).broadcast_to((P, G, E)),
                    ALU.mult,
                )
                wch = rtp.tile([P, G], fp32, tag="wch")
                nc.vector.reduce_sum(wch[:], w8[:], axis=AX.X)

                # token-major broadcast of the combine weight: [P, G] ->
                # transpose -> [1, TW] DRAM -> replicate to [P, TW]
                pswt = ps_t.tile([G, P], fp32)
                nc.tensor.transpose(pswt[:], wch[:], eye_sb[:])
                wrow = rtp.tile([G, P], fp32, tag="wrow")
                nc.scalar.copy(wrow[:], pswt[:])
                wcd = dramw.tile([1, TW], fp32)
                nc.sync.dma_start(wcd[0:1, :], wrow[:, :])
                wb = wbp.tile([P, TW], fp32)
                nc.sync.dma_start(wb[:], wcd[0:1, :].broadcast_to((P, TW)))

                # ---- mm1 + gelu: h.T chunk [FF, TW] in bf16 ----
                ht = htp.tile([P, FFC, TW], bf16)
                for ffc in range(FFC):
                    ph = ps_h.tile([P, TW], fp32)
                    for kc in range(KC):
                        nc.tensor.matmul(
                            ph[:],
                            w1_sb[:, kc, ffc * P:(ffc + 1) * P],
                            xb_sb[:, kc, :],
                            start=(kc == 0),
                            stop=(kc == KC - 1),
                        )
                    nc.scalar.activation(ht[:, ffc, :], ph[:], ACT.Gelu)

                # ---- mm2 + combine-weight scale: y.T chunk [DIM, TW] bf16 ----
                ysb = ytp.tile([P, DC, TW], bf16)
                for dc in range(DC):
                    py = ps_y.tile([P, TW], fp32)
                    for fc in range(FFC):
                        nc.tensor.matmul(
                            py[:],
                            w2_sb[:, fc, dc * P:(dc + 1) * P],
                            ht[:, fc, :],
                            start=(fc == 0),
                            stop=(fc == FFC - 1),
                        )
                    nc.vector.tensor_tensor(ysb[:, dc, :], py[:], wb[:], ALU.mult)
                for dc in range(DC):
                    nc.sync.dma_start(
                        yt2[dc * P:(dc + 1) * P, ts:ts + TW], ysb[:, dc, :]
                    )

            # ---- combine: ReduceScatter over the 8 expert contributions ----
            if collective:
                outr = dram.tile([P, N], bf16)
                nc.gpsimd.collective_compute(
                    "ReduceScatter",
                    mybir.AluOpType.add,
                    ins=[yt2[0:DIM, :].opt()],
                    outs=[outr.opt()],
                    replica_groups=[list(range(8))],
                )
                nc.sync.dma_start(out_ext[:, :], outr[:, :])
            else:
                nc.sync.dma_start(out_ext[:, :], yt2[0:P, :])

    _legalize_waits(nc)
    return nc


def kernel(x, Wr, W1, W2):
    import ml_dtypes
    from concourse.bass_utils import run_bass_kernel_spmd

    if "nc" not in _cache:
        _cache["nc"] = _build()
    nc = _cache["nc"]

    bf = ml_dtypes.bfloat16
    xf = x.reshape(N, DIM).astype(np.float32)
    xTf = np.ascontiguousarray(xf.T)
    xTb = xTf.astype(bf)
    wrtf = np.ascontiguousarray(Wr.T.astype(np.float32))
    eye = np.eye(P, dtype=np.float32)
    in_maps = []
    for c in range(8):
        esel = np.zeros((P, E), dtype=np.float32)
        esel[:, c] = 1.0
        in_maps.append({
            "xTf": xTf, "xTb": xTb, "wrt": wrtf,
            "w1": np.ascontiguousarray(W1[c]).astype(bf),
            "w2": np.ascontiguousarray(W2[c]).astype(bf),
            "esel": esel, "eye": eye,
        })
    res = run_bass_kernel_spmd(nc, in_maps, list(range(8)))
    _cache["last_result"] = res
    yT = np.concatenate(
        [res.results[c]["out"].astype(np.float32) for c in range(8)], axis=0
    )
    return np.ascontiguousarray(yT.T).reshape(B, T, DIM).astype(np.float32)
